# revision 25
# baseline (speedup 1.0000x reference)
"""Distributed Bass kernel for causal multi-head attention with RoPE.

Problem: B=2, S=2048, D=2048, H=16, HD=128 (nn_Attention_85315230368481).

Sharding: sequence-parallel over 8 cores with causal load balancing.
Per batch, the 2048 query rows form 16 blocks of 128; core c owns block
L=c and block H=15-c of each batch (512 rows total, local order
[b0L b1L b0H b1H]). Causality: an L block only attends to key blocks
0..7 and an H block to key blocks 0..15, so each core runs 8 "full"
steps (512 query cols) plus 8 "H-only" steps (256 cols) -- 25% less
score/attV/exp work than the unbalanced split, identical on every core
(per-core visibility differences live in the mask DATA, not program
structure).

Each core projects Q/K/V for its own rows, applies RoPE to Q and K,
AllGathers K^T and V across cores (bf16, 8 chunked collectives
interleaved with projection head-groups so the CC queue is busy from
~20us), then computes attention for its query rows and the output
projection. The host maps the 8 row-shards back to the full output.

Layout tricks:
 - x is passed transposed ([D, 512]) so Q^T/K^T ([head_dim, rows]) and
   V (natural [rows, D]) all come straight out of the PE array.
 - Wq/Wk columns are permuted per head (even dims then odd dims) so RoPE
   works on contiguous partition halves; scores are invariant to the
   permutation since both Q and K use it.
 - Scores are computed transposed ([keys, queries]) so exp(scores) is
   directly the moving operand of the attention*V matmul, and the
   softmax denominator is an accumulating ones-column matmul.
 - The mask add runs in-place on the score PSUM (only on columns that
   can be masked for some core), so exp reads PSUM directly.
 - Softmax normalization: DVE reciprocal on the PSUM denominator row,
   broadcast to 128 partitions on the (otherwise idle) GpSimd engine.
 - Weight streams ride the sync DMA queue; K^T/V attention tiles ride
   the gpsimd queue so collective-gated loads never block weights.
 - PSUM bank rule honored everywhere: only the first matmul touching a
   bank carries start=True (start clears has_written for the WHOLE
   bank; a second start would wipe sibling regions' accumulation).
 - All matmuls in bf16 (fp32 accumulation in PSUM).
"""

import sys

import ml_dtypes
import numpy as np

if "/opt/trn_rl_repo" not in sys.path:
    sys.path.insert(0, "/opt/trn_rl_repo")

B, S, D, H = 2, 2048, 2048, 16
HD = D // H            # 128
NCORES = 8
ROWS = 512             # local query rows per core (4 blocks of 128)
DCH = D // 128         # 16 contraction chunks
SCALE = 1.0 / float(np.sqrt(HD))
BF16 = ml_dtypes.bfloat16

_GRAPH = None
_TRACE = False
_LAST_EXEC_NS = None
_LAST_RES = None


def _build_graph():
    import concourse.mybir as mybir
    from concourse import bacc, tile

    f32 = mybir.dt.float32
    bf = mybir.dt.bfloat16
    Exp = mybir.ActivationFunctionType.Exp

    nc = bacc.Bacc("TRN2", target_bir_lowering=False, num_devices=NCORES)

    xT = nc.declare_dram_parameter("xT", [D, ROWS], bf, isOutput=False)
    wq = nc.declare_dram_parameter("wq", [D, D], bf, isOutput=False)
    wk = nc.declare_dram_parameter("wk", [D, D], bf, isOutput=False)
    wv = nc.declare_dram_parameter("wv", [D, D], bf, isOutput=False)
    wo = nc.declare_dram_parameter("wo", [D, D], bf, isOutput=False)
    cosT = nc.declare_dram_parameter("cosT", [HD // 2, ROWS], f32, isOutput=False)
    sinT = nc.declare_dram_parameter("sinT", [HD // 2, ROWS], f32, isOutput=False)
    maskL = nc.declare_dram_parameter("maskL", [1024, 256], f32, isOutput=False)
    maskH = nc.declare_dram_parameter("maskH", [1024, 256], f32, isOutput=False)
    onesd = nc.declare_dram_parameter("ones", [128, 128], bf, isOutput=False)
    out = nc.declare_dram_parameter("out", [ROWS, D], f32, isOutput=True)

    with nc.allow_low_precision(reason="bf16 matmul inputs; fp32 accumulate"), \
         tile.TileContext(nc) as tc:
        with (
            tc.tile_pool(name="dram", bufs=1, space="DRAM") as dramp,
            tc.tile_pool(name="const", bufs=1) as constp,
            tc.tile_pool(name="wstream", bufs=16) as wpool,
            tc.tile_pool(name="sbout", bufs=4) as sbout,
        ):
            k_in = dramp.tile([D, ROWS], bf)
            k_outs = [
                dramp.tile([NCORES * 512, ROWS], bf, addr_space="Shared",
                           name=f"k_out{i}")
                for i in range(4)
            ]
            v_ins = [
                dramp.tile([ROWS, 512], bf, name=f"v_in{i}") for i in range(4)
            ]
            v_outs = [
                dramp.tile([NCORES * ROWS, 512], bf, addr_space="Shared",
                           name=f"v_out{i}")
                for i in range(4)
            ]

            # resident tensors (loaded via the idle gpsimd DMA queue so the
            # sync queue can start streaming weights immediately)
            xts = constp.tile([128, DCH * ROWS], bf)         # x^T chunks
            for d in range(DCH):
                nc.gpsimd.dma_start(
                    out=xts[:, d * ROWS:(d + 1) * ROWS],
                    in_=xT[d * 128:(d + 1) * 128, :],
                )
            cos_sb = constp.tile([64, ROWS], f32)
            sin_sb = constp.tile([64, ROWS], f32)
            nc.gpsimd.dma_start(out=cos_sb[:], in_=cosT[:, :])
            nc.gpsimd.dma_start(out=sin_sb[:], in_=sinT[:, :])
            ones_sq = constp.tile([128, 128], bf)
            nc.gpsimd.dma_start(out=ones_sq[:], in_=onesd[:, :])
            mL_sb = constp.tile([128, 8 * 256], f32)
            mH_sb = constp.tile([128, 8 * 256], f32)
            for kb in range(8):
                nc.gpsimd.dma_start(
                    out=mL_sb[:, kb * 256:(kb + 1) * 256],
                    in_=maskL[kb * 128:(kb + 1) * 128, :],
                )
                nc.gpsimd.dma_start(
                    out=mH_sb[:, kb * 256:(kb + 1) * 256],
                    in_=maskH[kb * 128:(kb + 1) * 128, :],
                )

            qsb = constp.tile([128, H * ROWS], bf)           # rope'd Q^T per head
            attn = constp.tile([128, H * ROWS], bf)          # attention out^T per head

            def rope(dst, dst_cols, src_ps, tmp_pool):
                # src_ps: [128, ROWS] psum, rows 0:64 = even dims, 64:128 = odd
                dcs = slice(dst_cols, dst_cols + ROWS)
                te = src_ps[0:64, :]
                to = src_ps[64:128, :]
                t1 = tmp_pool.tile([64, ROWS], f32, tag="ropetmp1")
                t2 = tmp_pool.tile([64, ROWS], f32, tag="ropetmp2")
                nc.vector.tensor_mul(t1[:], te, cos_sb[:])
                nc.vector.tensor_mul(t2[:], to, sin_sb[:])
                nc.vector.tensor_sub(dst[0:64, dcs], t1[:], t2[:])
                t3 = tmp_pool.tile([64, ROWS], f32, tag="ropetmp3")
                t4 = tmp_pool.tile([64, ROWS], f32, tag="ropetmp4")
                nc.vector.tensor_mul(t3[:], te, sin_sb[:])
                nc.vector.tensor_mul(t4[:], to, cos_sb[:])
                nc.vector.tensor_add(dst[64:128, dcs], t3[:], t4[:])

            kpool = tc.alloc_tile_pool(name="ktiles", bufs=4)
            vpool = tc.alloc_tile_pool(name="vtiles", bufs=4)
            kta_t = {}
            vta_t = {}

            # rank-major views of the gathered K^T and V
            k_views = [
                k_outs[i][:].rearrange(
                    "(rk h d) c -> h d rk c", rk=NCORES, h=4, d=128
                )
                for i in range(4)
            ]
            v_views = [
                v_outs[i][:].rearrange(
                    "(cc p) (hh t) -> hh p cc t", cc=4 * NCORES, p=128, hh=4, t=128
                )
                for i in range(4)
            ]

            with (
                tc.tile_pool(name="projps", bufs=8, space="PSUM") as projps,
                tc.tile_pool(name="ropetmp", bufs=3) as ropep,
            ):
                # ---- interleaved per group g: K proj -> AG-K, Q proj, V proj -> AG-V
                for g in range(4):
                    # K projection + RoPE for heads 4g..4g+3
                    kps = [projps.tile([128, ROWS], f32, tag="projps", name="projtile")
                           for _ in range(4)]
                    for d in range(DCH):
                        wkt = wpool.tile([128, 512], bf, tag="wst")
                        nc.sync.dma_start(
                            out=wkt[:],
                            in_=wk[d * 128:(d + 1) * 128, g * 512:(g + 1) * 512],
                        )
                        for hh in range(4):
                            nc.tensor.matmul(
                                kps[hh][:],
                                lhsT=wkt[:, hh * 128:(hh + 1) * 128],
                                rhs=xts[:, d * ROWS:(d + 1) * ROWS],
                                start=(d == 0), stop=(d == DCH - 1),
                            )
                    for hh in range(4):
                        h = g * 4 + hh
                        ksb = sbout.tile([128, ROWS], bf, tag="ksb")
                        rope(ksb, 0, kps[hh], ropep)
                        nc.scalar.dma_start(
                            out=k_in[h * 128:(h + 1) * 128, :], in_=ksb[:]
                        )
                    nc.gpsimd.collective_compute(
                        "AllGather",
                        mybir.AluOpType.bypass,
                        replica_groups=[list(range(NCORES))],
                        ins=[k_in[g * 512:(g + 1) * 512, :].opt()],
                        outs=[k_outs[g].opt()],
                    )

                    # V projection for output cols 512g..512(g+1)
                    vps = [projps.tile([128, 512], f32, tag="projps", name="projtile")
                           for _ in range(4)]
                    for d in range(DCH):
                        wvt = wpool.tile([128, 512], bf, tag="wst")
                        nc.sync.dma_start(
                            out=wvt[:],
                            in_=wv[d * 128:(d + 1) * 128, g * 512:(g + 1) * 512],
                        )
                        for rr in range(4):
                            nc.tensor.matmul(
                                vps[rr][:],
                                lhsT=xts[:, d * ROWS + rr * 128:d * ROWS + (rr + 1) * 128],
                                rhs=wvt[:],
                                start=(d == 0), stop=(d == DCH - 1),
                            )
                    for rr in range(4):
                        vsb = sbout.tile([128, 512], bf, tag="vsb")
                        nc.scalar.copy(vsb[:], vps[rr][:])
                        nc.scalar.dma_start(
                            out=v_ins[g][rr * 128:(rr + 1) * 128, :],
                            in_=vsb[:],
                        )
                    nc.gpsimd.collective_compute(
                        "AllGather",
                        mybir.AluOpType.bypass,
                        replica_groups=[list(range(NCORES))],
                        ins=[v_ins[g].opt()],
                        outs=[v_outs[g].opt()],
                    )

                    # Q projection + RoPE for heads 4g..4g+3
                    qps = [projps.tile([128, ROWS], f32, tag="projps", name="projtile")
                           for _ in range(4)]
                    for d in range(DCH):
                        wqt = wpool.tile([128, 512], bf, tag="wst")
                        nc.sync.dma_start(
                            out=wqt[:],
                            in_=wq[d * 128:(d + 1) * 128, g * 512:(g + 1) * 512],
                        )
                        for hh in range(4):
                            nc.tensor.matmul(
                                qps[hh][:],
                                lhsT=wqt[:, hh * 128:(hh + 1) * 128],
                                rhs=xts[:, d * ROWS:(d + 1) * ROWS],
                                start=(d == 0), stop=(d == DCH - 1),
                            )
                    for hh in range(4):
                        h = g * 4 + hh
                        rope(qsb, h * ROWS, qps[hh], ropep)

                    if g == 0:
                        # prefetch K^T/V tiles for the first head group
                        for h in range(4):
                            kta_t[h] = kpool.tile([128, NCORES * ROWS], bf, tag="kt", name=f"kta{h}")
                            nc.gpsimd.dma_start(out=kta_t[h][:], in_=k_views[0][h])
                            vta_t[h] = vpool.tile([128, NCORES * ROWS], bf, tag="vt", name=f"vta{h}")
                            nc.gpsimd.dma_start(out=vta_t[h][:], in_=v_views[0][h])

            # ---- Attention per head ----
            # kta cols: key block (b, kb): kb<8 -> rank kb, col kb*512 + b*128
            #           kb>=8 -> rank 15-kb, col (15-kb)*512 + 256 + b*128
            # vta cols: cc*128, cc = rank*4 + local block (b0L,b1L,b0H,b1H)
            with (
                tc.tile_pool(name="scps", bufs=2, space="PSUM") as scps,
                tc.tile_pool(name="scHps", bufs=2, space="PSUM") as scHps,
                tc.tile_pool(name="attps", bufs=2, space="PSUM") as attps,
                tc.tile_pool(name="smallps", bufs=2, space="PSUM") as smallps,
                tc.tile_pool(name="extiles", bufs=6) as expool,
                tc.tile_pool(name="tmp", bufs=3) as tmpp,
            ):
                for h in range(H):
                    if h not in kta_t:
                        kta_t[h] = kpool.tile([128, NCORES * ROWS], bf, tag="kt", name=f"kta{h}")
                        nc.gpsimd.dma_start(out=kta_t[h][:], in_=k_views[h // 4][h % 4])
                        vta_t[h] = vpool.tile([128, NCORES * ROWS], bf, tag="vt", name=f"vta{h}")
                        nc.gpsimd.dma_start(out=vta_t[h][:], in_=v_views[h // 4][h % 4])
                    kta, vta = kta_t[h], vta_t[h]
                    qh = qsb[:, h * ROWS:(h + 1) * ROWS]
                    qv = qh.rearrange("p (s b c) -> p s b c", s=2, b=2, c=128)
                    # att2 cols: [b0L b0H b1L b1H]
                    att2 = attps.tile([128, 512], f32, tag="att2")
                    den = smallps.tile([1, 512], f32, tag="den")
                    rbs = tmpp.tile([128, 512], f32, tag="rbs")

                    # -- software-pipelined steps: produce scores/exp for
                    # step kb while the PE consumes (den+attV) step kb-1, so
                    # dependent matmuls never sit at the head of the PE queue
                    def produce(kb):
                        if kb < 8:
                            sc = scps.tile([128, 512], f32, tag="sc", name="sct")
                            sv = sc[:].rearrange("p (s b c) -> p s b c", s=2, b=2, c=128)
                            for b2 in range(B):
                                nc.tensor.matmul(
                                    sv[:, :, b2, :],
                                    lhsT=kta[:, kb * 512 + b2 * 128:kb * 512 + b2 * 128 + 128],
                                    rhs=qv[:, :, b2, :],
                                    start=(b2 == 0), stop=True,
                                )
                            nc.vector.tensor_add(
                                sc[:, 0:256], sc[:, 0:256],
                                mL_sb[:, kb * 256:(kb + 1) * 256],
                            )
                            ex = expool.tile([128, 512], bf, tag="ex", name="ext")
                            nc.scalar.activation(ex[:], sc[:], Exp, scale=SCALE)
                            return ex
                        scHt = scHps.tile([128, 512], f32, tag="scH", name="scHt")
                        scH = scHt[:, 0:256]
                        kcol = (15 - kb) * 512 + 256
                        for b2 in range(B):
                            nc.tensor.matmul(
                                scH[:, b2 * 128:(b2 + 1) * 128],
                                lhsT=kta[:, kcol + b2 * 128:kcol + b2 * 128 + 128],
                                rhs=qh[:, 256 + b2 * 128:256 + (b2 + 1) * 128],
                                start=(b2 == 0), stop=True,
                            )
                        nc.vector.tensor_add(
                            scH[:], scH[:],
                            mH_sb[:, (kb - 8) * 256:(kb - 7) * 256],
                        )
                        exH = expool.tile([128, 256], bf, tag="exH", name="exHt")
                        nc.scalar.activation(exH[:], scH[:], Exp, scale=SCALE)
                        return exH

                    def consume(kb, ex):
                        if kb < 7:
                            ev = ex[:].rearrange("p (s b c) -> p s b c", s=2, b=2, c=128)
                            nc.tensor.matmul(
                                den[:], lhsT=ones_sq[:, 0:1], rhs=ex[:],
                                start=(kb == 0), stop=False,
                            )
                            for b2 in range(B):
                                vcc = (kb * 4 + b2) * 128
                                nc.tensor.matmul(
                                    att2[:, b2 * 256:(b2 + 1) * 256],
                                    lhsT=vta[:, vcc:vcc + 128],
                                    rhs=ev[:, :, b2, :],
                                    start=(kb == 0 and b2 == 0), stop=False,
                                )
                        elif kb == 7:
                            nc.tensor.matmul(
                                den[0:1, 0:256], lhsT=ones_sq[:, 0:1],
                                rhs=ex[:, 0:256], start=False, stop=True,
                            )
                            nc.tensor.matmul(
                                den[0:1, 256:512], lhsT=ones_sq[:, 0:1],
                                rhs=ex[:, 256:512], start=False, stop=False,
                            )
                            for b2 in range(B):
                                vcc = (kb * 4 + b2) * 128
                                nc.tensor.matmul(
                                    att2[:, b2 * 256:b2 * 256 + 128],
                                    lhsT=vta[:, vcc:vcc + 128],
                                    rhs=ex[:, b2 * 128:(b2 + 1) * 128],
                                    start=False, stop=True,
                                )
                                nc.tensor.matmul(
                                    att2[:, b2 * 256 + 128:(b2 + 1) * 256],
                                    lhsT=vta[:, vcc:vcc + 128],
                                    rhs=ex[:, 256 + b2 * 128:256 + (b2 + 1) * 128],
                                    start=False, stop=False,
                                )
                        else:
                            nc.tensor.matmul(
                                den[0:1, 256:512], lhsT=ones_sq[:, 0:1], rhs=ex[:],
                                start=False, stop=(kb == 15),
                            )
                            for b2 in range(B):
                                vcc = ((15 - kb) * 4 + 2 + b2) * 128
                                nc.tensor.matmul(
                                    att2[:, b2 * 256 + 128:(b2 + 1) * 256],
                                    lhsT=vta[:, vcc:vcc + 128],
                                    rhs=ex[:, b2 * 128:(b2 + 1) * 128],
                                    start=False, stop=(kb == 15),
                                )

                    exq = None
                    for kb in range(16):
                        ex_new = produce(kb)
                        if exq is not None:
                            consume(kb - 1, exq)
                        exq = ex_new
                    consume(15, exq)

                    # -- normalize L blocks (their accumulation is complete) --
                    rcpL = tmpp.tile([1, 256], f32, tag="rcpL")
                    rscrL = tmpp.tile([1, 256], f32, tag="rscrL")
                    nc.vector.reciprocal_approx_accurate(
                        rcpL[:], den[0:1, 0:256], rscrL[:]
                    )
                    nc.gpsimd.partition_broadcast(rbs[:, 0:256], rcpL[:])
                    a2v = att2[:].rearrange("p (b s c) -> p b s c", b=2, s=2, c=128)
                    nc.vector.tensor_mul(
                        attn[:, h * ROWS:h * ROWS + 256],
                        a2v[:, :, 0, :], rbs[:, 0:256],
                    )

                    # -- normalize H blocks --
                    rcpH = tmpp.tile([1, 256], f32, tag="rcpH")
                    rscrH = tmpp.tile([1, 256], f32, tag="rscrH")
                    nc.vector.reciprocal_approx_accurate(
                        rcpH[:], den[0:1, 256:512], rscrH[:]
                    )
                    nc.gpsimd.partition_broadcast(rbs[:, 256:512], rcpH[:])
                    nc.vector.tensor_mul(
                        attn[:, h * ROWS + 256:(h + 1) * ROWS],
                        a2v[:, :, 1, :], rbs[:, 256:512],
                    )

            vpool.release()
            kpool.release()

            # ---- Output projection ----
            with tc.tile_pool(name="ops", bufs=8, space="PSUM") as opsp:
                for nn in range(4):
                    ops = [opsp.tile([128, 512], f32, tag="ops", name="opstile")
                           for _ in range(4)]
                    for h in range(H):
                        wot = wpool.tile([128, 512], bf, tag="wst")
                        nc.sync.dma_start(
                            out=wot[:],
                            in_=wo[h * 128:(h + 1) * 128, nn * 512:(nn + 1) * 512],
                        )
                        for qt in range(4):
                            nc.tensor.matmul(
                                ops[qt][:],
                                lhsT=attn[:, h * ROWS + qt * 128:h * ROWS + (qt + 1) * 128],
                                rhs=wot[:],
                                start=(h == 0), stop=(h == H - 1),
                            )
                    for qt in range(4):
                        osb = sbout.tile([128, 512], f32, tag="osb")
                        nc.scalar.copy(osb[:], ops[qt][:])
                        nc.sync.dma_start(
                            out=out[qt * 128:(qt + 1) * 128, nn * 512:(nn + 1) * 512],
                            in_=osb[:],
                        )

    nc.compile()
    return nc


def _get_graph():
    global _GRAPH
    if _GRAPH is None:
        _GRAPH = _build_graph()
    return _GRAPH


_PERM = np.concatenate(
    [h * HD + np.concatenate([np.arange(0, HD, 2), np.arange(1, HD, 2)])
     for h in range(H)]
)


def kernel(x, Wq, Wk, Wv, Wo, freqs_cos, freqs_sin, mask):
    global _LAST_EXEC_NS, _LAST_RES
    from concourse.bass_utils import run_bass_kernel_spmd

    nc = _get_graph()

    x = np.asarray(x, np.float32)
    wq_p = np.ascontiguousarray(np.asarray(Wq, np.float32)[:, _PERM]).astype(BF16)
    wk_p = np.ascontiguousarray(np.asarray(Wk, np.float32)[:, _PERM]).astype(BF16)
    wv_b = np.ascontiguousarray(np.asarray(Wv, np.float32)).astype(BF16)
    wo_b = np.ascontiguousarray(np.asarray(Wo, np.float32)).astype(BF16)
    cosf = np.asarray(freqs_cos, np.float32)
    sinf = np.asarray(freqs_sin, np.float32)
    maskf = np.asarray(mask, np.float32)[0, 0]      # [S, S] (q, k)
    ones_b = np.ones((128, 128), BF16)
    mscale = float(np.sqrt(HD))

    in_maps = []
    for c in range(NCORES):
        Lr = slice(c * 128, (c + 1) * 128)
        Hr = slice((15 - c) * 128, (16 - c) * 128)
        # local row order: [b0L b1L b0H b1H]
        x_c = np.concatenate(
            [x[0, Lr], x[1, Lr], x[0, Hr], x[1, Hr]], axis=0
        )  # [512, D]
        cos_c = np.concatenate([cosf[Lr], cosf[Lr], cosf[Hr], cosf[Hr]], axis=0)
        sin_c = np.concatenate([sinf[Lr], sinf[Lr], sinf[Hr], sinf[Hr]], axis=0)
        mLT = np.ascontiguousarray(maskf[Lr, 0:1024].T) * mscale    # [1024, 128]
        mHT = np.ascontiguousarray(maskf[Hr, 1024:2048].T) * mscale
        in_maps.append({
            "xT": np.ascontiguousarray(x_c.T).astype(BF16),
            "wq": wq_p, "wk": wk_p, "wv": wv_b, "wo": wo_b,
            "cosT": np.ascontiguousarray(cos_c.T),
            "sinT": np.ascontiguousarray(sin_c.T),
            "maskL": np.ascontiguousarray(np.concatenate([mLT, mLT], axis=1)),
            "maskH": np.ascontiguousarray(np.concatenate([mHT, mHT], axis=1)),
            "ones": ones_b,
        })

    res = run_bass_kernel_spmd(
        nc, in_maps, core_ids=list(range(NCORES)), trace=_TRACE,
    )
    _LAST_EXEC_NS = res.exec_time_ns
    _LAST_RES = res

    outp = np.empty((B, S, D), np.float32)
    for c in range(NCORES):
        o = res.results[c]["out"]
        Lr = slice(c * 128, (c + 1) * 128)
        Hr = slice((15 - c) * 128, (16 - c) * 128)
        outp[0, Lr] = o[0:128]
        outp[1, Lr] = o[128:256]
        outp[0, Hr] = o[256:384]
        outp[1, Hr] = o[384:512]
    return outp


# revision 28
# speedup vs baseline: 1.1055x; 1.1055x over previous
"""Distributed Bass kernel for causal multi-head attention with RoPE.

Problem: B=2, S=2048, D=2048, H=16, HD=128 (nn_Attention_85315230368481).

Sharding: sequence-parallel over 8 cores with causal load balancing.
Per batch, the 2048 query rows form 16 blocks of 128; core c owns block
L=c and block H=15-c of each batch (512 rows total, local order
[b0L b1L b0H b1H]). Causality: an L block only attends to key blocks
0..7 and an H block to key blocks 0..15, so each core runs 8 "full"
steps (512 query cols) plus 8 "H-only" steps (256 cols) -- 25% less
score/attV/exp work than the unbalanced split, identical on every core
(per-core visibility differences live in the mask DATA, not program
structure).

Each core projects Q/K/V for its own rows, applies RoPE to Q and K,
AllGathers K^T and V across cores (bf16, 8 chunked collectives
interleaved with projection head-groups so the CC queue is busy from
~20us), then computes attention for its query rows and the output
projection. The host maps the 8 row-shards back to the full output.

Layout tricks:
 - x is passed transposed ([D, 512]) so Q^T/K^T ([head_dim, rows]) and
   V (natural [rows, D]) all come straight out of the PE array.
 - Wq/Wk columns are permuted per head (even dims then odd dims) so RoPE
   works on contiguous partition halves; scores are invariant to the
   permutation since both Q and K use it.
 - Scores are computed transposed ([keys, queries]) so exp(scores) is
   directly the moving operand of the attention*V matmul, and the
   softmax denominator is an accumulating ones-column matmul.
 - The mask add runs in-place on the score PSUM (only on columns that
   can be masked for some core), so exp reads PSUM directly.
 - Softmax normalization: DVE reciprocal on the PSUM denominator row,
   broadcast to 128 partitions on the (otherwise idle) GpSimd engine.
 - Weight streams ride the sync DMA queue; K^T/V attention tiles ride
   the gpsimd queue so collective-gated loads never block weights.
 - PSUM bank rule honored everywhere: only the first matmul touching a
   bank carries start=True (start clears has_written for the WHOLE
   bank; a second start would wipe sibling regions' accumulation).
 - All matmuls in bf16 (fp32 accumulation in PSUM).
"""

import sys

import ml_dtypes
import numpy as np

if "/opt/trn_rl_repo" not in sys.path:
    sys.path.insert(0, "/opt/trn_rl_repo")

B, S, D, H = 2, 2048, 2048, 16
HD = D // H            # 128
NCORES = 8
ROWS = 512             # local query rows per core (4 blocks of 128)
DCH = D // 128         # 16 contraction chunks
SCALE = 1.0 / float(np.sqrt(HD))
BF16 = ml_dtypes.bfloat16

_GRAPH = None
_TRACE = False
_LAST_EXEC_NS = None
_LAST_RES = None


def _build_graph():
    import concourse.mybir as mybir
    from concourse import bacc, tile

    f32 = mybir.dt.float32
    bf = mybir.dt.bfloat16
    Exp = mybir.ActivationFunctionType.Exp

    nc = bacc.Bacc("TRN2", target_bir_lowering=False, num_devices=NCORES)

    xT = nc.declare_dram_parameter("xT", [D, ROWS], bf, isOutput=False)
    wq = nc.declare_dram_parameter("wq", [D, D], bf, isOutput=False)
    wk = nc.declare_dram_parameter("wk", [D, D], bf, isOutput=False)
    wv = nc.declare_dram_parameter("wv", [D, D], bf, isOutput=False)
    wo = nc.declare_dram_parameter("wo", [D, D], bf, isOutput=False)
    cosT = nc.declare_dram_parameter("cosT", [HD // 2, ROWS], f32, isOutput=False)
    sinT = nc.declare_dram_parameter("sinT", [HD // 2, ROWS], f32, isOutput=False)
    maskL = nc.declare_dram_parameter("maskL", [1024, 256], f32, isOutput=False)
    maskH = nc.declare_dram_parameter("maskH", [1024, 256], f32, isOutput=False)
    onesd = nc.declare_dram_parameter("ones", [128, 128], bf, isOutput=False)
    out = nc.declare_dram_parameter("out", [ROWS, D], f32, isOutput=True)

    with nc.allow_low_precision(reason="bf16 matmul inputs; fp32 accumulate"), \
         tile.TileContext(nc) as tc:
        with (
            tc.tile_pool(name="dram", bufs=1, space="DRAM") as dramp,
            tc.tile_pool(name="const", bufs=1) as constp,
            tc.tile_pool(name="wstream", bufs=16) as wpool,
            tc.tile_pool(name="sbout", bufs=4) as sbout,
        ):
            k_in = dramp.tile([D, ROWS], bf)
            k_outs = [
                dramp.tile([NCORES * 512, ROWS], bf, addr_space="Shared",
                           name=f"k_out{i}")
                for i in range(4)
            ]
            v_ins = [
                dramp.tile([ROWS, 512], bf, name=f"v_in{i}") for i in range(4)
            ]
            v_outs = [
                dramp.tile([NCORES * ROWS, 512], bf, addr_space="Shared",
                           name=f"v_out{i}")
                for i in range(4)
            ]

            # resident tensors (loaded via the idle gpsimd DMA queue so the
            # sync queue can start streaming weights immediately)
            xts = constp.tile([128, DCH * ROWS], bf)         # x^T chunks
            for d in range(DCH):
                nc.gpsimd.dma_start(
                    out=xts[:, d * ROWS:(d + 1) * ROWS],
                    in_=xT[d * 128:(d + 1) * 128, :],
                )
            cos_sb = constp.tile([64, ROWS], f32)
            sin_sb = constp.tile([64, ROWS], f32)
            nc.gpsimd.dma_start(out=cos_sb[:], in_=cosT[:, :])
            nc.gpsimd.dma_start(out=sin_sb[:], in_=sinT[:, :])
            ones_sq = constp.tile([128, 128], bf)
            nc.gpsimd.dma_start(out=ones_sq[:], in_=onesd[:, :])
            mL_sb = constp.tile([128, 8 * 256], f32)
            mH_sb = constp.tile([128, 8 * 256], f32)
            for kb in range(8):
                nc.gpsimd.dma_start(
                    out=mL_sb[:, kb * 256:(kb + 1) * 256],
                    in_=maskL[kb * 128:(kb + 1) * 128, :],
                )
                nc.gpsimd.dma_start(
                    out=mH_sb[:, kb * 256:(kb + 1) * 256],
                    in_=maskH[kb * 128:(kb + 1) * 128, :],
                )

            qsb = constp.tile([128, H * ROWS], bf)           # rope'd Q^T per head
            attn = constp.tile([128, H * ROWS], bf)          # attention out^T per head

            def rope(dst, dst_cols, src_ps, tmp_pool):
                # src_ps: [128, ROWS] psum, rows 0:64 = even dims, 64:128 = odd
                dcs = slice(dst_cols, dst_cols + ROWS)
                te = src_ps[0:64, :]
                to = src_ps[64:128, :]
                t1 = tmp_pool.tile([64, ROWS], f32, tag="ropetmp1")
                t2 = tmp_pool.tile([64, ROWS], f32, tag="ropetmp2")
                nc.vector.tensor_mul(t1[:], te, cos_sb[:])
                nc.vector.tensor_mul(t2[:], to, sin_sb[:])
                nc.vector.tensor_sub(dst[0:64, dcs], t1[:], t2[:])
                t3 = tmp_pool.tile([64, ROWS], f32, tag="ropetmp3")
                t4 = tmp_pool.tile([64, ROWS], f32, tag="ropetmp4")
                nc.vector.tensor_mul(t3[:], te, sin_sb[:])
                nc.vector.tensor_mul(t4[:], to, cos_sb[:])
                nc.vector.tensor_add(dst[64:128, dcs], t3[:], t4[:])

            kpool = tc.alloc_tile_pool(name="ktiles", bufs=4)
            vpool = tc.alloc_tile_pool(name="vtiles", bufs=4)
            kta_t = {}
            vta_t = {}

            # rank-major views of the gathered K^T and V
            k_views = [
                k_outs[i][:].rearrange(
                    "(rk h d) c -> h d rk c", rk=NCORES, h=4, d=128
                )
                for i in range(4)
            ]
            v_views = [
                v_outs[i][:].rearrange(
                    "(cc p) (hh t) -> hh p cc t", cc=4 * NCORES, p=128, hh=4, t=128
                )
                for i in range(4)
            ]

            with (
                tc.tile_pool(name="projps", bufs=8, space="PSUM") as projps,
                tc.tile_pool(name="ropetmp", bufs=3) as ropep,
            ):
                # ---- interleaved per group g: K proj -> AG-K, Q proj, V proj -> AG-V
                for g in range(4):
                    # K projection + RoPE for heads 4g..4g+3
                    kps = [projps.tile([128, ROWS], f32, tag="projps", name="projtile")
                           for _ in range(4)]
                    for d in range(DCH):
                        wkt = wpool.tile([128, 512], bf, tag="wst")
                        nc.sync.dma_start(
                            out=wkt[:],
                            in_=wk[d * 128:(d + 1) * 128, g * 512:(g + 1) * 512],
                        )
                        for hh in range(4):
                            nc.tensor.matmul(
                                kps[hh][:],
                                lhsT=wkt[:, hh * 128:(hh + 1) * 128],
                                rhs=xts[:, d * ROWS:(d + 1) * ROWS],
                                start=(d == 0), stop=(d == DCH - 1),
                            )
                    for hh in range(4):
                        h = g * 4 + hh
                        ksb = sbout.tile([128, ROWS], bf, tag="ksb")
                        rope(ksb, 0, kps[hh], ropep)
                        nc.scalar.dma_start(
                            out=k_in[h * 128:(h + 1) * 128, :], in_=ksb[:]
                        )
                    nc.gpsimd.collective_compute(
                        "AllGather",
                        mybir.AluOpType.bypass,
                        replica_groups=[list(range(NCORES))],
                        ins=[k_in[g * 512:(g + 1) * 512, :].opt()],
                        outs=[k_outs[g].opt()],
                    )

                    # Q projection + RoPE for heads 4g..4g+3
                    qps = [projps.tile([128, ROWS], f32, tag="projps", name="projtile")
                           for _ in range(4)]
                    for d in range(DCH):
                        wqt = wpool.tile([128, 512], bf, tag="wst")
                        nc.sync.dma_start(
                            out=wqt[:],
                            in_=wq[d * 128:(d + 1) * 128, g * 512:(g + 1) * 512],
                        )
                        for hh in range(4):
                            nc.tensor.matmul(
                                qps[hh][:],
                                lhsT=wqt[:, hh * 128:(hh + 1) * 128],
                                rhs=xts[:, d * ROWS:(d + 1) * ROWS],
                                start=(d == 0), stop=(d == DCH - 1),
                            )
                    for hh in range(4):
                        h = g * 4 + hh
                        rope(qsb, h * ROWS, qps[hh], ropep)

                    # V projection for output cols 512g..512(g+1)
                    vps = [projps.tile([128, 512], f32, tag="projps", name="projtile")
                           for _ in range(4)]
                    for d in range(DCH):
                        wvt = wpool.tile([128, 512], bf, tag="wst")
                        nc.sync.dma_start(
                            out=wvt[:],
                            in_=wv[d * 128:(d + 1) * 128, g * 512:(g + 1) * 512],
                        )
                        for rr in range(4):
                            nc.tensor.matmul(
                                vps[rr][:],
                                lhsT=xts[:, d * ROWS + rr * 128:d * ROWS + (rr + 1) * 128],
                                rhs=wvt[:],
                                start=(d == 0), stop=(d == DCH - 1),
                            )
                    for rr in range(4):
                        vsb = sbout.tile([128, 512], bf, tag="vsb")
                        nc.scalar.copy(vsb[:], vps[rr][:])
                        nc.scalar.dma_start(
                            out=v_ins[g][rr * 128:(rr + 1) * 128, :],
                            in_=vsb[:],
                        )
                    nc.gpsimd.collective_compute(
                        "AllGather",
                        mybir.AluOpType.bypass,
                        replica_groups=[list(range(NCORES))],
                        ins=[v_ins[g].opt()],
                        outs=[v_outs[g].opt()],
                    )

                    if g == 0:
                        # prefetch K^T/V tiles for the first head group
                        for h in range(4):
                            kta_t[h] = kpool.tile([128, NCORES * ROWS], bf, tag="kt", name=f"kta{h}")
                            nc.gpsimd.dma_start(out=kta_t[h][:], in_=k_views[0][h])
                            vta_t[h] = vpool.tile([128, NCORES * ROWS], bf, tag="vt", name=f"vta{h}")
                            nc.gpsimd.dma_start(out=vta_t[h][:], in_=v_views[0][h])

            # ---- Attention per head ----
            # kta cols: key block (b, kb): kb<8 -> rank kb, col kb*512 + b*128
            #           kb>=8 -> rank 15-kb, col (15-kb)*512 + 256 + b*128
            # vta cols: cc*128, cc = rank*4 + local block (b0L,b1L,b0H,b1H)
            with (
                tc.tile_pool(name="scps", bufs=2, space="PSUM") as scps,
                tc.tile_pool(name="scHps", bufs=2, space="PSUM") as scHps,
                tc.tile_pool(name="attps", bufs=2, space="PSUM") as attps,
                tc.tile_pool(name="smallps", bufs=2, space="PSUM") as smallps,
                tc.tile_pool(name="extiles", bufs=6) as expool,
                tc.tile_pool(name="tmp", bufs=3) as tmpp,
            ):
                for h in range(H):
                    if h not in kta_t:
                        kta_t[h] = kpool.tile([128, NCORES * ROWS], bf, tag="kt", name=f"kta{h}")
                        nc.gpsimd.dma_start(out=kta_t[h][:], in_=k_views[h // 4][h % 4])
                        vta_t[h] = vpool.tile([128, NCORES * ROWS], bf, tag="vt", name=f"vta{h}")
                        nc.gpsimd.dma_start(out=vta_t[h][:], in_=v_views[h // 4][h % 4])
                    kta, vta = kta_t[h], vta_t[h]
                    qh = qsb[:, h * ROWS:(h + 1) * ROWS]
                    qv = qh.rearrange("p (s b c) -> p s b c", s=2, b=2, c=128)
                    # att2 cols: [b0L b0H b1L b1H]
                    att2 = attps.tile([128, 512], f32, tag="att2")
                    den = smallps.tile([1, 512], f32, tag="den")
                    rbs = tmpp.tile([128, 512], f32, tag="rbs")

                    # -- software-pipelined steps: produce scores/exp for
                    # step kb while the PE consumes (den+attV) step kb-1, so
                    # dependent matmuls never sit at the head of the PE queue
                    def produce(kb):
                        if kb < 8:
                            sc = scps.tile([128, 512], f32, tag="sc", name="sct")
                            sv = sc[:].rearrange("p (s b c) -> p s b c", s=2, b=2, c=128)
                            for b2 in range(B):
                                nc.tensor.matmul(
                                    sv[:, :, b2, :],
                                    lhsT=kta[:, kb * 512 + b2 * 128:kb * 512 + b2 * 128 + 128],
                                    rhs=qv[:, :, b2, :],
                                    start=(b2 == 0), stop=True,
                                )
                            nc.vector.tensor_add(
                                sc[:, 0:256], sc[:, 0:256],
                                mL_sb[:, kb * 256:(kb + 1) * 256],
                            )
                            ex = expool.tile([128, 512], bf, tag="ex", name="ext")
                            nc.scalar.activation(ex[:], sc[:], Exp, scale=SCALE)
                            return ex
                        scHt = scHps.tile([128, 512], f32, tag="scH", name="scHt")
                        scH = scHt[:, 0:256]
                        kcol = (15 - kb) * 512 + 256
                        for b2 in range(B):
                            nc.tensor.matmul(
                                scH[:, b2 * 128:(b2 + 1) * 128],
                                lhsT=kta[:, kcol + b2 * 128:kcol + b2 * 128 + 128],
                                rhs=qh[:, 256 + b2 * 128:256 + (b2 + 1) * 128],
                                start=(b2 == 0), stop=True,
                            )
                        nc.vector.tensor_add(
                            scH[:], scH[:],
                            mH_sb[:, (kb - 8) * 256:(kb - 7) * 256],
                        )
                        exH = expool.tile([128, 256], bf, tag="exH", name="exHt")
                        nc.scalar.activation(exH[:], scH[:], Exp, scale=SCALE)
                        return exH

                    def consume(kb, ex):
                        if kb < 7:
                            ev = ex[:].rearrange("p (s b c) -> p s b c", s=2, b=2, c=128)
                            nc.tensor.matmul(
                                den[:], lhsT=ones_sq[:, 0:1], rhs=ex[:],
                                start=(kb == 0), stop=False,
                            )
                            for b2 in range(B):
                                vcc = (kb * 4 + b2) * 128
                                nc.tensor.matmul(
                                    att2[:, b2 * 256:(b2 + 1) * 256],
                                    lhsT=vta[:, vcc:vcc + 128],
                                    rhs=ev[:, :, b2, :],
                                    start=(kb == 0 and b2 == 0), stop=False,
                                )
                        elif kb == 7:
                            nc.tensor.matmul(
                                den[0:1, 0:256], lhsT=ones_sq[:, 0:1],
                                rhs=ex[:, 0:256], start=False, stop=True,
                            )
                            nc.tensor.matmul(
                                den[0:1, 256:512], lhsT=ones_sq[:, 0:1],
                                rhs=ex[:, 256:512], start=False, stop=False,
                            )
                            for b2 in range(B):
                                vcc = (kb * 4 + b2) * 128
                                nc.tensor.matmul(
                                    att2[:, b2 * 256:b2 * 256 + 128],
                                    lhsT=vta[:, vcc:vcc + 128],
                                    rhs=ex[:, b2 * 128:(b2 + 1) * 128],
                                    start=False, stop=True,
                                )
                                nc.tensor.matmul(
                                    att2[:, b2 * 256 + 128:(b2 + 1) * 256],
                                    lhsT=vta[:, vcc:vcc + 128],
                                    rhs=ex[:, 256 + b2 * 128:256 + (b2 + 1) * 128],
                                    start=False, stop=False,
                                )
                        else:
                            nc.tensor.matmul(
                                den[0:1, 256:512], lhsT=ones_sq[:, 0:1], rhs=ex[:],
                                start=False, stop=(kb == 15),
                            )
                            for b2 in range(B):
                                vcc = ((15 - kb) * 4 + 2 + b2) * 128
                                nc.tensor.matmul(
                                    att2[:, b2 * 256 + 128:(b2 + 1) * 256],
                                    lhsT=vta[:, vcc:vcc + 128],
                                    rhs=ex[:, b2 * 128:(b2 + 1) * 128],
                                    start=False, stop=(kb == 15),
                                )

                    exq = None
                    for kb in range(16):
                        ex_new = produce(kb)
                        if exq is not None:
                            consume(kb - 1, exq)
                        exq = ex_new
                    consume(15, exq)

                    # -- normalize L blocks (their accumulation is complete) --
                    rcpL = tmpp.tile([1, 256], f32, tag="rcpL")
                    rscrL = tmpp.tile([1, 256], f32, tag="rscrL")
                    nc.vector.reciprocal_approx_accurate(
                        rcpL[:], den[0:1, 0:256], rscrL[:]
                    )
                    nc.gpsimd.partition_broadcast(rbs[:, 0:256], rcpL[:])
                    a2v = att2[:].rearrange("p (b s c) -> p b s c", b=2, s=2, c=128)
                    nc.vector.tensor_mul(
                        attn[:, h * ROWS:h * ROWS + 256],
                        a2v[:, :, 0, :], rbs[:, 0:256],
                    )

                    # -- normalize H blocks --
                    rcpH = tmpp.tile([1, 256], f32, tag="rcpH")
                    rscrH = tmpp.tile([1, 256], f32, tag="rscrH")
                    nc.vector.reciprocal_approx_accurate(
                        rcpH[:], den[0:1, 256:512], rscrH[:]
                    )
                    nc.gpsimd.partition_broadcast(rbs[:, 256:512], rcpH[:])
                    nc.vector.tensor_mul(
                        attn[:, h * ROWS + 256:(h + 1) * ROWS],
                        a2v[:, :, 1, :], rbs[:, 256:512],
                    )

            vpool.release()
            kpool.release()

            # ---- Output projection ----
            with tc.tile_pool(name="ops", bufs=8, space="PSUM") as opsp:
                for nn in range(4):
                    ops = [opsp.tile([128, 512], f32, tag="ops", name="opstile")
                           for _ in range(4)]
                    for h in range(H):
                        wot = wpool.tile([128, 512], bf, tag="wst")
                        nc.sync.dma_start(
                            out=wot[:],
                            in_=wo[h * 128:(h + 1) * 128, nn * 512:(nn + 1) * 512],
                        )
                        for qt in range(4):
                            nc.tensor.matmul(
                                ops[qt][:],
                                lhsT=attn[:, h * ROWS + qt * 128:h * ROWS + (qt + 1) * 128],
                                rhs=wot[:],
                                start=(h == 0), stop=(h == H - 1),
                            )
                    for qt in range(4):
                        osb = sbout.tile([128, 512], f32, tag="osb")
                        nc.scalar.copy(osb[:], ops[qt][:])
                        nc.sync.dma_start(
                            out=out[qt * 128:(qt + 1) * 128, nn * 512:(nn + 1) * 512],
                            in_=osb[:],
                        )

    nc.compile()
    return nc


def _get_graph():
    global _GRAPH
    if _GRAPH is None:
        _GRAPH = _build_graph()
    return _GRAPH


_PERM = np.concatenate(
    [h * HD + np.concatenate([np.arange(0, HD, 2), np.arange(1, HD, 2)])
     for h in range(H)]
)


def kernel(x, Wq, Wk, Wv, Wo, freqs_cos, freqs_sin, mask):
    global _LAST_EXEC_NS, _LAST_RES
    from concourse.bass_utils import run_bass_kernel_spmd

    nc = _get_graph()

    x = np.asarray(x, np.float32)
    wq_p = np.ascontiguousarray(np.asarray(Wq, np.float32)[:, _PERM]).astype(BF16)
    wk_p = np.ascontiguousarray(np.asarray(Wk, np.float32)[:, _PERM]).astype(BF16)
    wv_b = np.ascontiguousarray(np.asarray(Wv, np.float32)).astype(BF16)
    wo_b = np.ascontiguousarray(np.asarray(Wo, np.float32)).astype(BF16)
    cosf = np.asarray(freqs_cos, np.float32)
    sinf = np.asarray(freqs_sin, np.float32)
    maskf = np.asarray(mask, np.float32)[0, 0]      # [S, S] (q, k)
    ones_b = np.ones((128, 128), BF16)
    mscale = float(np.sqrt(HD))

    in_maps = []
    for c in range(NCORES):
        Lr = slice(c * 128, (c + 1) * 128)
        Hr = slice((15 - c) * 128, (16 - c) * 128)
        # local row order: [b0L b1L b0H b1H]
        x_c = np.concatenate(
            [x[0, Lr], x[1, Lr], x[0, Hr], x[1, Hr]], axis=0
        )  # [512, D]
        cos_c = np.concatenate([cosf[Lr], cosf[Lr], cosf[Hr], cosf[Hr]], axis=0)
        sin_c = np.concatenate([sinf[Lr], sinf[Lr], sinf[Hr], sinf[Hr]], axis=0)
        mLT = np.ascontiguousarray(maskf[Lr, 0:1024].T) * mscale    # [1024, 128]
        mHT = np.ascontiguousarray(maskf[Hr, 1024:2048].T) * mscale
        in_maps.append({
            "xT": np.ascontiguousarray(x_c.T).astype(BF16),
            "wq": wq_p, "wk": wk_p, "wv": wv_b, "wo": wo_b,
            "cosT": np.ascontiguousarray(cos_c.T),
            "sinT": np.ascontiguousarray(sin_c.T),
            "maskL": np.ascontiguousarray(np.concatenate([mLT, mLT], axis=1)),
            "maskH": np.ascontiguousarray(np.concatenate([mHT, mHT], axis=1)),
            "ones": ones_b,
        })

    res = run_bass_kernel_spmd(
        nc, in_maps, core_ids=list(range(NCORES)), trace=_TRACE,
    )
    _LAST_EXEC_NS = res.exec_time_ns
    _LAST_RES = res

    outp = np.empty((B, S, D), np.float32)
    for c in range(NCORES):
        o = res.results[c]["out"]
        Lr = slice(c * 128, (c + 1) * 128)
        Hr = slice((15 - c) * 128, (16 - c) * 128)
        outp[0, Lr] = o[0:128]
        outp[1, Lr] = o[128:256]
        outp[0, Hr] = o[256:384]
        outp[1, Hr] = o[384:512]
    return outp
